# revision 3
# baseline (speedup 1.0000x reference)
"""Trainium2 Bass kernel for a single-step attention LSTM decoder (batch=1).

Model (see reference): embedding lookup -> additive-attention over encoder
outputs -> combine+relu -> LSTM cell -> vocab projection -> log_softmax.

Distribution over 8 NeuronCores (tensor parallel):
  - emb_W replicated; each core gathers the embedded row via indirect DMA.
  - attention stage replicated on every core (tiny).
  - comb_W output-sharded (each core computes its 256 lstm_in dims).
  - [W_ih | W_hh] contraction-sharded (each core contracts over its own
    lstm_in/h0 slice) -> partial gates -> AllReduce(add) -> full gates on
    every core -> LSTM elementwise replicated -> full h_new everywhere.
  - out_W vocab-sharded (6400 padded rows per core) -> per-core logits shard
    -> local max/sumexp stats -> AllGather of (max, sum) pairs -> global
    log-normalizer computed locally -> per-core logp shard, host concat.

All matvecs use the PE in "stationary weight" form: out[128,1] chunks =
lhsT[128k,128m].T @ v[128,1], accumulated over contraction chunks in PSUM,
so every vector lives in a [128, n_chunk] column layout (col j = elements
j*128..j*128+127). The embedded row is gathered as [128, 16] with partition-
major element order; weight matrices contracting against it are row-permuted
on the host to match.

Weights stream from HBM in large contiguous tiles; that stream is the
roofline (~73MB/core fp32, ~360GB/s/core).
"""

import os
import numpy as np
import ml_dtypes

import concourse.bass as bass
import concourse.bacc as bacc
import concourse.tile as tile
import concourse.mybir as mybir
import concourse.bass_utils as bass_utils

F32 = mybir.dt.float32
BF16 = mybir.dt.bfloat16
I32 = mybir.dt.int32
AF = mybir.ActivationFunctionType
ALU = mybir.AluOpType
AX = mybir.AxisListType

NC_N = 8
H = 2048
V = 50257
L = 80
G4 = 4 * H            # 8192
HS = H // NC_N        # 256 per-core hidden shard
VS = 6400             # per-core (padded) vocab shard; 8*6400 = 51200 >= V
NEG = -1.0e9          # logit padding bias

# weight dtype: "f32" or "bf16"
W_MODE = os.environ.get("BASS_DEC_WDT", "bf16")


# ----------------------------------------------------------------- program --

def build_program(w_mode: str = W_MODE):
    WDT = F32 if w_mode == "f32" else BF16
    npw = np.float32 if w_mode == "f32" else ml_dtypes.bfloat16

    nc = bacc.Bacc("TRN2", target_bir_lowering=False, debug=False,
                   enable_asserts=False, num_devices=NC_N)

    # ---- inputs (per-core DRAM tensors) ----
    emb = nc.dram_tensor("emb", [V * 128, 16], BF16, kind="ExternalInput")
    xf = nc.dram_tensor("xf", [1, 1], F32, kind="ExternalInput")
    h_cols = nc.dram_tensor("h_cols", [128, 16], WDT, kind="ExternalInput")
    h_loc = nc.dram_tensor("h_loc", [128, 2], WDT, kind="ExternalInput")
    c_cols = nc.dram_tensor("c_cols", [128, 16], F32, kind="ExternalInput")
    enc = nc.dram_tensor("enc", [L, H], WDT, kind="ExternalInput")
    attn_wt = nc.dram_tensor("attn_wt", [2 * H, L], WDT, kind="ExternalInput")
    attn_b = nc.dram_tensor("attn_b", [L, 1], F32, kind="ExternalInput")
    comb_wt = nc.dram_tensor("comb_wt", [2 * H, HS], WDT, kind="ExternalInput")
    comb_b_loc = nc.dram_tensor("comb_b_loc", [128, 2], F32, kind="ExternalInput")
    wg = nc.dram_tensor("wg", [2 * HS, G4], WDT, kind="ExternalInput")
    bih = nc.dram_tensor("bih", [128, 64], F32, kind="ExternalInput")
    bhh = nc.dram_tensor("bhh", [128, 64], F32, kind="ExternalInput")
    out_wt = nc.dram_tensor("out_wt", [H, VS], WDT, kind="ExternalInput")
    out_b = nc.dram_tensor("out_b", [128, VS // 128], F32, kind="ExternalInput")
    iota_in = nc.dram_tensor("iota_in", [128, 1], F32, kind="ExternalInput")
    ones_in = nc.dram_tensor("ones_in", [1, 128], F32, kind="ExternalInput")
    ident_in = nc.dram_tensor("ident_in", [128, 128], F32, kind="ExternalInput")

    # ---- outputs ----
    logp_s = nc.dram_tensor("logp_s", [128, VS // 128], F32, kind="ExternalOutput")
    attn_row = nc.dram_tensor("attn_row", [1, L], F32, kind="ExternalOutput")

    NM = VS // 128  # 50 output m-chunks per core

    with tile.TileContext(nc) as tc:
        with (
            tc.tile_pool(name="small", bufs=1) as sp,
            tc.tile_pool(name="wpool", bufs=2) as wp,
            tc.tile_pool(name="opool", bufs=2) as op,
            tc.tile_pool(name="ps", bufs=1, space="PSUM") as ps,
            tc.tile_pool(name="pst", bufs=1, space="PSUM") as pst,
            tc.tile_pool(name="dram", bufs=1, space="DRAM") as dp,
        ):
            # ---------------- small resident loads ----------------
            def load(name, dram_t, shape, dtype):
                t = sp.tile(shape, dtype, tag=name)
                nc.sync.dma_start(out=t[:], in_=dram_t.ap())
                return t

            xf_sb = load("xf", xf, [1, 1], F32)
            iota_sb = load("iota", iota_in, [128, 1], F32)
            ones_sb = load("ones", ones_in, [1, 128], F32)
            ident_sb = load("ident", ident_in, [128, 128], F32)
            h_cols_sb = load("h_cols", h_cols, [128, 16], WDT)
            h_loc_sb = load("h_loc", h_loc, [128, 2], WDT)
            c_cols_sb = load("c_cols", c_cols, [128, 16], F32)
            attn_b_sb = load("attn_b", attn_b, [L, 1], F32)
            comb_b_sb = load("comb_b", comb_b_loc, [128, 2], F32)
            bih_sb = load("bih", bih, [128, 64], F32)
            bhh_sb = load("bhh", bhh, [128, 64], F32)
            out_b_sb = load("out_b", out_b, [128, NM], F32)

            enc_sb = sp.tile([L, H], WDT, tag="enc")
            nc.sync.dma_start(out=enc_sb[:], in_=enc.ap())
            attn_w_sb = sp.tile([128, 32 * L], WDT, tag="attn_w")
            nc.sync.dma_start(
                out=attn_w_sb[:].rearrange("p (c m) -> p c m", m=L),
                in_=attn_wt.ap().rearrange("(c p) m -> p c m", p=128))
            comb_w_sb = sp.tile([128, 32 * HS], WDT, tag="comb_w")
            nc.sync.dma_start(
                out=comb_w_sb[:].rearrange("p (c m) -> p c m", m=HS),
                in_=comb_wt.ap().rearrange("(c p) m -> p c m", p=128))

            # ---------------- embedding gather ----------------
            xb_ps = pst.tile([128, 1], F32, tag="pstmp")
            nc.tensor.matmul(out=xb_ps[:], lhsT=ones_sb[:], rhs=xf_sb[:],
                             start=True, stop=True)
            idx_f = sp.tile([128, 1], F32, tag="idx_f")
            nc.vector.scalar_tensor_tensor(
                out=idx_f[:], in0=xb_ps[:], scalar=128.0, in1=iota_sb[:],
                op0=ALU.mult, op1=ALU.add)
            idx_i = sp.tile([128, 1], I32, tag="idx_i")
            nc.vector.tensor_copy(out=idx_i[:], in_=idx_f[:])
            emb_bf = sp.tile([128, 16], BF16, tag="emb_bf")
            nc.gpsimd.indirect_dma_start(
                out=emb_bf[:], out_offset=None, in_=emb.ap(),
                in_offset=bass.IndirectOffsetOnAxis(ap=idx_i[:, :1], axis=0))
            if WDT == BF16:
                emb_sb = emb_bf
            else:
                emb_sb = sp.tile([128, 16], F32, tag="emb_f32")
                nc.vector.tensor_copy(out=emb_sb[:], in_=emb_bf[:])

            # ---------------- attention logits ----------------
            # cat_he = [h0 (chunks 0..15, col layout) ; embedded (chunks 16..31, p-major)]
            att_ps = ps.tile([L, 1], F32, tag="att_ps")
            for c in range(32):
                rhs = h_cols_sb[:, c:c + 1] if c < 16 else emb_sb[:, c - 16:c - 15]
                nc.tensor.matmul(out=att_ps[:], lhsT=attn_w_sb[:, c * L:(c + 1) * L],
                                 rhs=rhs, start=(c == 0), stop=(c == 31))
            att_l = sp.tile([L, 1], F32, tag="att_l")
            nc.vector.tensor_add(out=att_l[:], in0=att_ps[:], in1=attn_b_sb[:])

            # softmax over the 80 partitions: transpose to a row
            attr_ps = pst.tile([1, L], F32, tag="pstmp_row")
            nc.tensor.transpose(out=attr_ps[:], in_=att_l[:],
                                identity=ident_sb[:L, :L])
            att_row = sp.tile([1, L], F32, tag="att_row")
            nc.vector.tensor_copy(out=att_row[:], in_=attr_ps[:])
            amax = sp.tile([1, 1], F32, tag="amax")
            nc.vector.tensor_reduce(out=amax[:], in_=att_row[:], axis=AX.X, op=ALU.max)
            namax = sp.tile([1, 1], F32, tag="namax")
            nc.vector.tensor_scalar_mul(namax[:], amax[:], -1.0)
            aexp = sp.tile([1, L], F32, tag="aexp")
            asum = sp.tile([1, 1], F32, tag="asum")
            nc.scalar.activation(out=aexp[:], in_=att_row[:], func=AF.Exp,
                                 bias=namax[:, 0:1], scale=1.0, accum_out=asum[:, 0:1])
            rsum = sp.tile([1, 1], F32, tag="rsum")
            nc.vector.reciprocal(out=rsum[:], in_=asum[:])
            aw_row = sp.tile([1, L], F32, tag="aw_row")
            nc.vector.tensor_scalar(out=aw_row[:], in0=aexp[:], scalar1=rsum[:, 0:1],
                                    scalar2=None, op0=ALU.mult)
            nc.sync.dma_start(out=attn_row.ap(), in_=aw_row[:])

            # transpose attn weights back to a column for the PE
            awc_ps = pst.tile([L, 1], F32, tag="pstmp")
            nc.tensor.transpose(out=awc_ps[:], in_=aw_row[:],
                                identity=ones_sb[:1, :1])
            aw_col = sp.tile([L, 1], WDT, tag="aw_col")
            nc.vector.tensor_copy(out=aw_col[:], in_=awc_ps[:])

            # ---------------- attn_applied = attn_w @ encoder_outputs ----------------
            app_ps = ps.tile([128, 16], F32, tag="app_ps")
            for m in range(16):
                nc.tensor.matmul(out=app_ps[:, m:m + 1],
                                 lhsT=enc_sb[:, m * 128:(m + 1) * 128],
                                 rhs=aw_col[:], start=True, stop=True)
            app_sb = sp.tile([128, 16], WDT, tag="app_sb")
            nc.vector.tensor_copy(out=app_sb[:], in_=app_ps[:])

            # ---------------- lstm_in shard = relu(comb_W_k @ [emb; app] + b) ----------
            li_ps = ps.tile([128, 2], F32, tag="li_ps")
            for c in range(32):
                rhs = emb_sb[:, c:c + 1] if c < 16 else app_sb[:, c - 16:c - 15]
                for m in range(2):
                    nc.tensor.matmul(
                        out=li_ps[:, m:m + 1],
                        lhsT=comb_w_sb[:, c * HS + m * 128: c * HS + (m + 1) * 128],
                        rhs=rhs, start=(c == 0 and m == 0),
                        stop=(c == 31 and m == 1), skip_group_check=True)
            li_sb = sp.tile([128, 2], F32, tag="li_sb")
            for m in range(2):
                nc.scalar.activation(out=li_sb[:, m:m + 1], in_=li_ps[:, m:m + 1],
                                     func=AF.Relu, bias=comb_b_sb[:, m:m + 1], scale=1.0)
            if WDT == BF16:
                li_w = sp.tile([128, 2], BF16, tag="li_w")
                nc.vector.tensor_copy(out=li_w[:], in_=li_sb[:])
            else:
                li_w = li_sb

            # ---------------- partial gates = Wg_k.T-slices @ [lstm_in_k; h0_k] -------
            g_ps = ps.tile([128, 64], F32, tag="g_ps")
            for c in range(4):
                wgt = wp.tile([128, G4], WDT, tag="wg_tile")
                nc.sync.dma_start(out=wgt[:], in_=wg.ap()[c * 128:(c + 1) * 128, :])
                rhs = li_w[:, c:c + 1] if c < 2 else h_loc_sb[:, c - 2:c - 1]
                for m in range(64):
                    nc.tensor.matmul(out=g_ps[:, m:m + 1],
                                     lhsT=wgt[:, m * 128:(m + 1) * 128],
                                     rhs=rhs, start=(c == 0 and m == 0),
                                     stop=(c == 3 and m == 63), skip_group_check=True)
            g_part = sp.tile([128, 64], F32, tag="g_part")
            nc.vector.tensor_copy(out=g_part[:], in_=g_ps[:])

            # ---------------- AllReduce gates ----------------
            ar_in = dp.tile([128, 64], F32)
            ar_out = dp.tile([128, 64], F32)
            nc.gpsimd.dma_start(out=ar_in[:], in_=g_part[:])
            nc.gpsimd.collective_compute(
                "AllReduce", ALU.add, replica_groups=[list(range(NC_N))],
                ins=[ar_in[:]], outs=[ar_out[:]])
            gates = sp.tile([128, 64], F32, tag="gates")
            nc.gpsimd.dma_start(out=gates[:], in_=ar_out[:])
            nc.vector.tensor_add(out=gates[:], in0=gates[:], in1=bih_sb[:])
            nc.vector.tensor_add(out=gates[:], in0=gates[:], in1=bhh_sb[:])

            # ---------------- LSTM cell (full, replicated) ----------------
            sig_i = sp.tile([128, 16], F32, tag="sig_i")
            sig_f = sp.tile([128, 16], F32, tag="sig_f")
            tanh_g = sp.tile([128, 16], F32, tag="tanh_g")
            sig_o = sp.tile([128, 16], F32, tag="sig_o")
            nc.scalar.activation(out=sig_i[:], in_=gates[:, 0:16], func=AF.Sigmoid)
            nc.scalar.activation(out=sig_f[:], in_=gates[:, 16:32], func=AF.Sigmoid)
            nc.scalar.activation(out=tanh_g[:], in_=gates[:, 32:48], func=AF.Tanh)
            nc.scalar.activation(out=sig_o[:], in_=gates[:, 48:64], func=AF.Sigmoid)
            c_new = sp.tile([128, 16], F32, tag="c_new")
            nc.vector.tensor_mul(out=c_new[:], in0=sig_f[:], in1=c_cols_sb[:])
            ig = sp.tile([128, 16], F32, tag="ig")
            nc.vector.tensor_mul(out=ig[:], in0=sig_i[:], in1=tanh_g[:])
            nc.vector.tensor_add(out=c_new[:], in0=c_new[:], in1=ig[:])
            tanh_c = sp.tile([128, 16], F32, tag="tanh_c")
            nc.scalar.activation(out=tanh_c[:], in_=c_new[:], func=AF.Tanh)
            h_new = sp.tile([128, 16], F32, tag="h_new")
            nc.vector.tensor_mul(out=h_new[:], in0=sig_o[:], in1=tanh_c[:])
            if WDT == BF16:
                h_new_w = sp.tile([128, 16], BF16, tag="h_new_w")
                nc.vector.tensor_copy(out=h_new_w[:], in_=h_new[:])
            else:
                h_new_w = h_new

            # ---------------- vocab projection shard ----------------
            o_ps = ps.tile([128, NM], F32, tag="o_ps")
            for c in range(16):
                owt = op.tile([128, VS], WDT, tag="out_tile")
                nc.sync.dma_start(out=owt[:], in_=out_wt.ap()[c * 128:(c + 1) * 128, :])
                rhs = h_new_w[:, c:c + 1]
                for m in range(NM):
                    nc.tensor.matmul(out=o_ps[:, m:m + 1],
                                     lhsT=owt[:, m * 128:(m + 1) * 128],
                                     rhs=rhs, start=(c == 0 and m == 0),
                                     stop=(c == 15 and m == NM - 1),
                                     skip_group_check=True)
            logits = sp.tile([128, NM], F32, tag="logits")
            nc.vector.tensor_add(out=logits[:], in0=o_ps[:], in1=out_b_sb[:])

            # ---------------- local log-softmax stats ----------------
            mx1 = sp.tile([128, 1], F32, tag="mx1")
            nc.vector.tensor_reduce(out=mx1[:], in_=logits[:], axis=AX.X, op=ALU.max)
            mxr_ps = pst.tile([1, 128], F32, tag="pstmp_row")
            nc.tensor.transpose(out=mxr_ps[:], in_=mx1[:], identity=ident_sb[:])
            mx_row = sp.tile([1, 128], F32, tag="mx_row")
            nc.vector.tensor_copy(out=mx_row[:], in_=mxr_ps[:])
            m_loc = sp.tile([1, 1], F32, tag="m_loc")
            nc.vector.tensor_reduce(out=m_loc[:], in_=mx_row[:], axis=AX.X, op=ALU.max)
            # broadcast -m_loc to 128 partitions
            mb_ps = pst.tile([128, 1], F32, tag="pstmp")
            nc.tensor.matmul(out=mb_ps[:], lhsT=ones_sb[:], rhs=m_loc[:],
                             start=True, stop=True)
            negm = sp.tile([128, 1], F32, tag="negm")
            nc.scalar.mul(negm[:], mb_ps[:], -1.0)
            etile = sp.tile([128, NM], F32, tag="etile")
            s1 = sp.tile([128, 1], F32, tag="s1")
            nc.scalar.activation(out=etile[:], in_=logits[:], func=AF.Exp,
                                 bias=negm[:, 0:1], scale=1.0, accum_out=s1[:, 0:1])
            sr_ps = pst.tile([1, 128], F32, tag="pstmp_row")
            nc.tensor.transpose(out=sr_ps[:], in_=s1[:], identity=ident_sb[:])
            s_row = sp.tile([1, 128], F32, tag="s_row")
            nc.vector.tensor_copy(out=s_row[:], in_=sr_ps[:])
            s_loc = sp.tile([1, 1], F32, tag="s_loc")
            nc.vector.tensor_reduce(out=s_loc[:], in_=s_row[:], axis=AX.X, op=ALU.add)

            stat = sp.tile([1, 2], F32, tag="stat")
            nc.vector.tensor_copy(out=stat[:, 0:1], in_=m_loc[:])
            nc.vector.tensor_copy(out=stat[:, 1:2], in_=s_loc[:])
            ag_in = dp.tile([1, 2], F32)
            ag_out = dp.tile([1, 2 * NC_N], F32)
            nc.gpsimd.dma_start(out=ag_in[:], in_=stat[:])
            nc.gpsimd.collective_compute(
                "AllGather", ALU.bypass, replica_groups=[list(range(NC_N))],
                ins=[ag_in[:]], outs=[ag_out[:]])
            st_sb = sp.tile([1, 2 * NC_N], F32, tag="st_sb")
            nc.gpsimd.dma_start(out=st_sb[:], in_=ag_out[:])

            # logZ = g + log(sum_j s_j * exp(m_j - g)),  g = max_j m_j
            mv = st_sb[:].rearrange("p (r t) -> p t r", t=2)
            g1 = sp.tile([1, 1], F32, tag="g1")
            nc.vector.tensor_reduce(out=g1[:], in_=mv[:, 0:1, :], axis=AX.X, op=ALU.max)
            d8 = sp.tile([1, NC_N], F32, tag="d8")
            nc.vector.tensor_scalar(out=d8[:], in0=mv[:, 0:1, :], scalar1=g1[:, 0:1],
                                    scalar2=None, op0=ALU.subtract)
            e8 = sp.tile([1, NC_N], F32, tag="e8")
            nc.scalar.activation(out=e8[:], in_=d8[:], func=AF.Exp)
            sv = sp.tile([1, NC_N], F32, tag="sv")
            nc.vector.tensor_copy(out=sv[:], in_=mv[:, 1:2, :])
            w8 = sp.tile([1, NC_N], F32, tag="w8")
            nc.vector.tensor_mul(out=w8[:], in0=e8[:], in1=sv[:])
            tot = sp.tile([1, 1], F32, tag="tot")
            nc.vector.tensor_reduce(out=tot[:], in_=w8[:], axis=AX.X, op=ALU.add)
            lt = sp.tile([1, 1], F32, tag="lt")
            nc.scalar.activation(out=lt[:], in_=tot[:], func=AF.Ln)
            logz = sp.tile([1, 1], F32, tag="logz")
            nc.vector.tensor_add(out=logz[:], in0=lt[:], in1=g1[:])
            zb_ps = pst.tile([128, 1], F32, tag="pstmp")
            nc.tensor.matmul(out=zb_ps[:], lhsT=ones_sb[:], rhs=logz[:],
                             start=True, stop=True)
            negz = sp.tile([128, 1], F32, tag="negz")
            nc.scalar.mul(negz[:], zb_ps[:], -1.0)
            logp_sb = sp.tile([128, NM], F32, tag="logp_sb")
            nc.vector.tensor_scalar(out=logp_sb[:], in0=logits[:],
                                    scalar1=negz[:, 0:1], scalar2=None, op0=ALU.add)
            nc.sync.dma_start(out=logp_s.ap(), in_=logp_sb[:])

    nc.compile()
    return nc


# ------------------------------------------------------------- host prep --

def prep_in_maps(inputs: dict, w_mode: str = W_MODE):
    npw = np.float32 if w_mode == "f32" else ml_dtypes.bfloat16
    f32 = np.float32

    def cols(vec, n):  # (n*128,) -> [128, n] column-chunk layout
        return np.ascontiguousarray(np.asarray(vec, f32).reshape(n, 128).T)

    x = int(np.asarray(inputs["x"]).ravel()[0])
    emb_W = np.asarray(inputs["emb_W"], f32)
    enc_np = np.asarray(inputs["encoder_outputs"], f32)
    h0 = np.asarray(inputs["h"], f32).reshape(H)
    c0 = np.asarray(inputs["c"], f32).reshape(H)
    attn_W = np.asarray(inputs["attn_W"], f32)     # (L, 2H)
    attn_b = np.asarray(inputs["attn_b"], f32)     # (L,)
    comb_W = np.asarray(inputs["comb_W"], f32)     # (H, 2H)
    comb_b = np.asarray(inputs["comb_b"], f32)     # (H,)
    W_ih = np.asarray(inputs["W_ih"], f32)         # (4H, H)
    W_hh = np.asarray(inputs["W_hh"], f32)         # (4H, H)
    b_ih = np.asarray(inputs["b_ih"], f32)
    b_hh = np.asarray(inputs["b_hh"], f32)
    out_W = np.asarray(inputs["out_W"], f32)       # (V, H)
    out_b = np.asarray(inputs["out_b"], f32)       # (V,)

    def pmajor_perm(wt):
        # rows of wt are natural contraction order q (q = p*16 + j); reorder to
        # j*128 + p so they line up with the gathered [128,16] embedded tile.
        d = wt.shape[1]
        return wt.reshape(128, 16, d).transpose(1, 0, 2).reshape(2048, d)

    # replicated tensors
    emb_r = np.ascontiguousarray(emb_W.astype(npw if False else ml_dtypes.bfloat16)
                                 ).reshape(V * 128, 16)
    xf = np.array([[float(x)]], f32)
    h_cols = cols(h0, 16).astype(npw)
    c_cols = cols(c0, 16)
    enc_r = enc_np.astype(npw)
    attn_wt = np.concatenate([
        np.ascontiguousarray(attn_W[:, :H].T),            # h part, natural
        pmajor_perm(np.ascontiguousarray(attn_W[:, H:].T)),  # emb part, p-major
    ], axis=0).astype(npw)
    attn_b_r = attn_b.reshape(L, 1)
    bih_r = cols(b_ih, 64)
    bhh_r = cols(b_hh, 64)
    iota = np.arange(128, dtype=f32).reshape(128, 1)
    ones = np.ones((1, 128), f32)
    ident = np.eye(128, dtype=f32)

    # padded vocab shards
    VPAD = NC_N * VS
    out_W_p = np.zeros((VPAD, H), f32)
    out_W_p[:V] = out_W
    out_b_p = np.full((VPAD,), NEG, f32)
    out_b_p[:V] = out_b

    in_maps = []
    for k in range(NC_N):
        sl = slice(k * HS, (k + 1) * HS)
        comb_wt = np.concatenate([
            pmajor_perm(np.ascontiguousarray(comb_W[sl, :H].T)),  # emb part
            np.ascontiguousarray(comb_W[sl, H:].T),               # app part
        ], axis=0).astype(npw)
        wg_k = np.ascontiguousarray(
            np.concatenate([W_ih[:, sl], W_hh[:, sl]], axis=1).T).astype(npw)
        vsl = slice(k * VS, (k + 1) * VS)
        out_wt_k = np.ascontiguousarray(out_W_p[vsl].T).astype(npw)
        out_b_k = np.ascontiguousarray(out_b_p[vsl].reshape(VS // 128, 128).T)
        in_maps.append(dict(
            emb=emb_r, xf=xf, h_cols=h_cols, h_loc=np.ascontiguousarray(h_cols[:, 2 * k:2 * k + 2]),
            c_cols=c_cols, enc=enc_r, attn_wt=attn_wt, attn_b=attn_b_r,
            comb_wt=comb_wt, comb_b_loc=np.ascontiguousarray(cols(comb_b, 16)[:, 2 * k:2 * k + 2]),
            wg=wg_k, bih=bih_r, bhh=bhh_r, out_wt=out_wt_k, out_b=out_b_k,
            iota_in=iota, ones_in=ones, ident_in=ident,
        ))
    return in_maps


def assemble_outputs(results):
    shards = []
    for k in range(NC_N):
        lp = np.asarray(results[k]["logp_s"], np.float32)   # [128, 50]
        shards.append(lp.T.reshape(VS))                      # v = m*128 + p
    logp = np.concatenate(shards)[:V].reshape(1, V)
    attn = np.asarray(results[0]["attn_row"], np.float32).reshape(1, 1, L)
    return logp, attn


# --------------------------------------------------------------- entry ----

_CACHE = {}


def _get_program(w_mode: str = W_MODE):
    if w_mode not in _CACHE:
        _CACHE[w_mode] = build_program(w_mode)
    return _CACHE[w_mode]


def kernel(**inputs):
    nc = _get_program()
    in_maps = prep_in_maps(inputs)
    res = bass_utils.run_bass_kernel_spmd(
        nc, in_maps, core_ids=list(range(NC_N)), trace=False)
    return assemble_outputs(res.results)
